# revision 39
# baseline (speedup 1.0000x reference)
"""Trainium2 Bass kernel for a single transformer block (nn_Block_3212635537783).

Reference computation (B=4, T=2048, C=768, H=12, D=64):
    q/k/v per-head projections of x; scores[t,s] = k[t]@q[s]/sqrt(C) with
    causal mask (s <= t), softmax over s; a[t] = sum_s w[t,s] v[s];
    x = LN1(x + a); x = LN2(x + gelu(x@W1 + b1)@W2 + b2)

Sharding: 8 cores = 4 batches x 2 token-interleaved halves. Core (b, g)
owns rows {g, g+2, ...} of batch b. The stride-2 interleave keeps the
causal workload balanced AND the SPMD program identical across cores
(only input data differs; the +-1 row causal boundary lives in a tiny
per-core mask tile).

On-chip layout is fully "transposed": activations are [C, tokens]
(feature dim on partitions) so attention, layernorm and the MLP never
need an on-chip transpose.

Precision: q/k/v projections and the A@V matmul run in fp8e4 with
DoubleRow perf mode (256-wide contraction per instruction, 2x PE
throughput); weights are pre-scaled x16 into fp8's sweet spot and
unscaled in the PSUM->SBUF copies. Scores stay bf16 (64-deep
contraction gains nothing from DoubleRow). The MLP stays bf16
(fp8 there costs ~2e-2 relative error; measured off-line).
Softmax denominators ride the A@V matmul as a 65th "ones" value row;
their reciprocals are batched 4-heads-at-a-time into one DVE op via
32-aligned partition-shifted row extraction.
"""

import sys
import types

import numpy as np
import ml_dtypes

B, T, C, H, D = 4, 2048, 768, 12, 64
F = 4 * C            # 3072
P = 128              # partitions
OT = T // 2          # owned tokens per core (1024)
NB_C = C // P        # 6 c-chunks
NB_CP = NB_C // 2    # 3 c-pair chunks (fp8 DoubleRow contraction)
NB_F = F // P        # 24 hidden chunks
NPAIR = H // 2       # 6 head-pair chunks
EPS = 1e-5
SCALE = float(1.0 / np.sqrt(np.float32(C)))
WSCL = 16.0          # fp8 weight pre-scale
N_CORES = 8
HG = 4               # heads per attention group
N_HG = H // HG       # 3 groups

BF16 = ml_dtypes.bfloat16
E4M3 = ml_dtypes.float8_e4m3

_compiled = {}


# --------------------------------------------------------------------------
# environment patches (must live in kernel.py: the grader imports only this
# file). Idempotent.
# --------------------------------------------------------------------------

def _patch_tile_drain():
    """This walrus build rejects >1 sync-wait command on the final Tile
    drain CTRL instruction. Spread the drain's waits across chained
    sync-engine nops (same engine => program order preserved; the
    all-engine barrier after them still gates the semaphore clears)."""
    import concourse.tile as tile_mod
    import concourse.mybir as mybir

    if getattr(tile_mod.TileContext, "_drain_patched", False):
        return

    def patched(self, tick_clock, wait_clock):
        from concourse.vector_clock import ScopedClock

        drain_inst = self.nc.sync.drain()
        wait_clock.add_sem_waits(
            drain_inst.ins, ScopedClock({None: tick_clock.global_clock})
        )
        si = drain_inst.ins.sync_info
        waits = list(si.on_wait) if si else []
        MAXW = 1
        if len(waits) > MAXW:
            si.on_wait = waits[:MAXW]
            rest = waits[MAXW:]
            while rest:
                nop = self.nc.sync.nop(nofuse=True)
                chunk, rest = rest[:MAXW], rest[MAXW:]
                nsi = nop.ins.sync_info
                if nsi is None:
                    nop.ins.sync_info = mybir.SyncInfo(on_wait=chunk, on_update=[])
                else:
                    nsi.on_wait = list(nsi.on_wait) + chunk
        self.nc.all_engine_barrier()
        assert self.sems is not None
        popped = self.nc._tile_sem_poison_stack.pop()
        assert popped is self._sem_poison
        self.nc.clear_and_free_semaphores(list(self.sems.allocated().values()))
        self.nc.all_engine_barrier()

    tile_mod.TileContext._drain_and_barrier = patched
    tile_mod.TileContext._drain_patched = True


def _patch_profile_hook():
    """Optional: register the axon NTFF profiling hook so trace=True works
    (used for timing; harmless no-op if unavailable)."""
    if "antenv.axon_hooks" in sys.modules:
        return
    try:
        sys.path.insert(0, "/root/.axon_site")
        from trn_agent_boot.trn_boot import _ntff_profile_via_ctypes

        hook = _ntff_profile_via_ctypes("/opt/axon/libaxon_pjrt.so")
        mod = types.ModuleType("antenv.axon_hooks")
        mod.get_axon_ntff_profile_hook = lambda: hook
        mod.set_axon_ntff_profile_hook = lambda h: None
        sys.modules["antenv.axon_hooks"] = mod
        import concourse.bass_utils as bu

        bu.upload_artifacts = lambda tmpdir: "local://" + tmpdir
    except Exception:
        pass


# --------------------------------------------------------------------------
# program construction (shared by all 8 cores; SPMD over input data)
# --------------------------------------------------------------------------

def _build_nc():
    import contextlib

    import concourse.bass as bass
    import concourse.mybir as mybir
    from concourse.tile import TileContext

    f32 = mybir.dt.float32
    f32r = mybir.dt.float32r
    bf16 = mybir.dt.bfloat16
    fp8 = mybir.dt.float8e4
    ALU = mybir.AluOpType
    AF = mybir.ActivationFunctionType
    PM = mybir.MatmulPerfMode

    nc = bass.Bass()

    # ---- DRAM I/O ----
    # fp8 x^T in c-pair DoubleRow layout: [cpair, partition, j, token]
    x8d = nc.declare_dram_parameter("x8d", [NB_CP, P, 2, T], fp8, isOutput=False)
    x8od = nc.declare_dram_parameter("x8od", [NB_CP, P, 2, OT], fp8, isOutput=False)
    # bf16 own-token x^T (attention residual): [partition, cchunk, token]
    xo16d = nc.declare_dram_parameter("xo16d", [P, NB_C, OT], bf16, isOutput=False)
    # fp8 x16-scaled qkv weights, c-pair layout [partition, cpair, j, outcol]
    # ((cpair, j) merge into one DMA dim; 3-dim AP limit)
    wq8d = nc.declare_dram_parameter("wq8d", [P, NB_CP, 2, C], fp8, isOutput=False)
    wk8d = nc.declare_dram_parameter("wk8d", [P, NB_CP, 2, C], fp8, isOutput=False)
    wv8d = nc.declare_dram_parameter("wv8d", [P, NB_CP, 2, C], fp8, isOutput=False)
    w1d = nc.declare_dram_parameter("w1d", [C, F], bf16, isOutput=False)
    w2d = nc.declare_dram_parameter("w2d", [F, C], bf16, isOutput=False)
    b1r = nc.declare_dram_parameter("b1r", [P, NB_F], f32, isOutput=False)
    b2r = nc.declare_dram_parameter("b2r", [P, NB_C], f32, isOutput=False)
    g1r = nc.declare_dram_parameter("g1r", [P, NB_C], f32, isOutput=False)
    be1r = nc.declare_dram_parameter("be1r", [P, NB_C], f32, isOutput=False)
    g2r = nc.declare_dram_parameter("g2r", [P, NB_C], f32, isOutput=False)
    be2r = nc.declare_dram_parameter("be2r", [P, NB_C], f32, isOutput=False)
    # boundary masks: cmask [P, 64] ({0,1} stripe); cmask2 [P, 128]
    # (64 zero cols ++ the same stripe) for odd chunks of fp8 AV pairs
    cmask = nc.declare_dram_parameter("cmask", [P, 64], bf16, isOutput=False)
    cmask2 = nc.declare_dram_parameter("cmask2", [P, 128], bf16, isOutput=False)
    outT = nc.declare_dram_parameter("outT", [C, OT], f32, isOutput=True)

    w1_t = w1d[:].rearrange("(n p) f -> p n f", p=P)
    w2_t = w2d[:].rearrange("(m p) c -> p m c", p=P)
    outT_t = outT[:].rearrange("(n p) t -> n p t", p=P)

    with TileContext(nc) as tc, contextlib.ExitStack() as ctx:
        const = ctx.enter_context(tc.tile_pool(name="const", bufs=1))
        p_res = ctx.enter_context(tc.tile_pool(name="res", bufs=1))
        p_a = ctx.enter_context(tc.tile_pool(name="attn_a", bufs=1))
        p_mlpw = ctx.enter_context(tc.tile_pool(name="mlpw", bufs=1))
        p_dn = ctx.enter_context(tc.tile_pool(name="dn", bufs=1))
        import contextlib as _ctl
        xt_stack = _ctl.ExitStack()
        p_xt = xt_stack.enter_context(tc.tile_pool(name="xt", bufs=1))

        # ---- constants (DMAs issued later, after hot weight loads) ----
        ones_k = const.tile([P, 1], bf16, tag="ones_k", name="ones_k")
        nc.vector.memset(ones_k, 1.0)
        ones_bf = const.tile([1, P], f32, tag="ones_bf", name="ones_bf")
        nc.vector.memset(ones_bf, 1.0)
        ones_b = const.tile([1, P], f32r, tag="ones_b", name="ones_b")
        with nc.allow_low_precision(reason="f32r ones for 1cyc/row bcast"):
            nc.vector.tensor_copy(ones_b, ones_bf)
        # all-partition ones (f32r): rank-1 broadcast lhsT taken at the same
        # partition offset as the rhs row it pairs with
        ones_f = const.tile([P, 64], f32, tag="ones_f", name="ones_f")
        nc.vector.memset(ones_f, 1.0)
        ones_p = const.tile([P, 64], f32r, tag="ones_p", name="ones_p")
        with nc.allow_low_precision(reason="f32r ones for 1cyc/row bcast"):
            nc.vector.tensor_copy(ones_p, ones_f)
        eps_t = const.tile([1, 1], f32, tag="eps", name="eps_t")
        nc.vector.memset(eps_t, EPS)
        msk = const.tile([P, 64], bf16, tag="msk", name="msk")
        msk2 = bass.AP(
            tensor=msk.tensor, offset=msk.offset,
            ap=[list(msk.ap[0]), [0, 2], list(msk.ap[1])],
        )
        mskw = const.tile([P, 128], bf16, tag="mskw", name="mskw")
        mskw2 = bass.AP(
            tensor=mskw.tensor, offset=mskw.offset,
            ap=[list(mskw.ap[0]), [0, 2], list(mskw.ap[1])],
        )
        sb_b1 = const.tile([P, NB_F], f32, tag="b1", name="sb_b1")
        sb_b2 = const.tile([P, NB_C], f32, tag="b2", name="sb_b2")
        sb_g1 = const.tile([P, NB_C], f32, tag="g1", name="sb_g1")
        sb_be1 = const.tile([P, NB_C], f32, tag="be1", name="sb_be1")
        sb_g2 = const.tile([P, NB_C], f32, tag="g2", name="sb_g2")
        sb_be2 = const.tile([P, NB_C], f32, tag="be2", name="sb_be2")

        def load_consts():
            nc.sync.dma_start(out=msk, in_=cmask[:])
            nc.sync.dma_start(out=mskw, in_=cmask2[:])
            nc.sync.dma_start(out=sb_b1, in_=b1r[:])
            nc.sync.dma_start(out=sb_b2, in_=b2r[:])
            nc.sync.dma_start(out=sb_g1, in_=g1r[:])
            nc.sync.dma_start(out=sb_be1, in_=be1r[:])
            nc.sync.dma_start(out=sb_g2, in_=g2r[:])
            nc.sync.dma_start(out=sb_be2, in_=be2r[:])

        # ---- persistent activations ----
        # fp8 x^T per c-pair: [128, 2, T]; serves as DoubleRow rhs for q/k
        # (j, t slices) AND DoubleRow lhsT for v (j, token-chunk slices).
        sb_x8 = [
            p_xt.tile([P, 2, T], fp8, tag=f"x8_{cp}", name=f"x8_{cp}")
            for cp in range(NB_CP)
        ]
        sb_x8o = [
            p_xt.tile([P, 2, OT], fp8, tag=f"x8o_{cp}", name=f"x8o_{cp}")
            for cp in range(NB_CP)
        ]
        # attention output a^T, bf16 [128, OT] per pair-chunk
        sb_a = [
            p_a.tile([P, OT], bf16, tag=f"a{pc}", name=f"a{pc}")
            for pc in range(NPAIR)
        ]

        # MLP weights: single packed tiles, one DMA each (issued after the
        # group-0 projections; earlier would queue 9.4MB ahead of hot loads)
        sb_w1 = p_mlpw.tile([P, NB_C, F], bf16, tag="w1", name="w1")
        sb_w2 = p_mlpw.tile([P, NB_F, C], bf16, tag="w2", name="w2")

        # softmax-normalize state (p_dn is global: the apply step is
        # deferred into the NEXT group's projection phase, where PE is
        # dense and spare psum rotation slots exist)
        pending_norm = []

        def apply_norm_gen(pool, tag):
            """Deferred normalize part 2: per head, PE rank-1 broadcast of
            the reciprocal row, DVE psum->sbuf copy, bf16 DVE multiply.
            Yields every 2 heads so the broadcasts spread across the
            interleaved stream. Only flushes entries pending at entry
            (more may be appended while suspended)."""
            snap = list(pending_norm)
            del pending_norm[: len(snap)]
            for ent in snap:
                for hj, h in enumerate(ent["heads"]):
                    pc, par = h // 2, h % 2
                    den_ps = pool.tile(
                        [64, 512], f32, tag=tag, bufs=2, name="den_ps"
                    )
                    if hj < 3:
                        nc.tensor.matmul(
                            den_ps,
                            ones_p[32 * hj : 32 * hj + 1, :],
                            ent["rec4"][32 * hj : 32 * hj + 1, :],
                            start=True, stop=True,
                        )
                    else:
                        nc.tensor.matmul(
                            den_ps, ones_p[0:1, :], ent["rec_h3"],
                            start=True, stop=True,
                        )
                    den_sb = p_dn.tile(
                        [64, 512], bf16, tag=f"den_sb{hj}", bufs=2,
                        name=f"den_sb{hj}",
                    )
                    nc.vector.tensor_copy(den_sb, den_ps)
                    nc.vector.tensor_tensor(
                        sb_a[pc][par * 64 : par * 64 + 64,
                                 ent["tb"] * 512 : (ent["tb"] + 1) * 512],
                        ent["av_sb"][h],
                        den_sb,
                        ALU.mult,
                    )
                    if hj % 2 == 1:
                        yield

        # ============================================================
        # Phase A: attention, in head groups of HG. Group hg+1's
        # projections are INTERLEAVED into group hg's attention stream:
        # PE executes in order, so attention matmuls waiting on ACT exps
        # would otherwise block the independent projection matmuls
        # queued behind them (head-of-line bubbles).
        # ============================================================
        gen_stack = _ctl.ExitStack()
        p_w = gen_stack.enter_context(tc.tile_pool(name="wqkv", bufs=1))
        p_qk = gen_stack.enter_context(tc.tile_pool(name="qk", bufs=1))
        p_v = gen_stack.enter_context(tc.tile_pool(name="vv", bufs=1))
        p_e = gen_stack.enter_context(tc.tile_pool(name="ee", bufs=1))
        p_ps = gen_stack.enter_context(
            tc.tile_pool(name="aps", bufs=1, space="PSUM")
        )

        def dma_wg(tile, src, col0):
            # src [p, cp, j, col] -> tile [p, cp, j, 256]
            nc.sync.dma_start(
                out=tile, in_=src[:][:, :, :, col0 : col0 + 2 * P]
            )

        def prep_group(hg):
            """Allocate group tiles (generation-rotated tags) and issue
            the group's weight DMAs."""
            pcs = [hg * (HG // 2) + i for i in range(HG // 2)]
            heads = [2 * pc + j for pc in pcs for j in range(2)]
            d0 = heads[0] * D
            st = dict(hg=hg, pcs=pcs, heads=heads)
            st["wq8"] = p_w.tile([P, NB_CP, 2, 2 * P], fp8, tag="wq8",
                                 bufs=2, name=f"wq8_{hg}")
            st["wk8"] = p_w.tile([P, NB_CP, 2, 2 * P], fp8, tag="wk8",
                                 bufs=2, name=f"wk8_{hg}")
            st["wv8"] = p_w.tile([P, NB_CP, 2, 2 * P], fp8, tag="wv8",
                                 bufs=2, name=f"wv8_{hg}")
            dma_wg(st["wq8"], wq8d, d0)
            dma_wg(st["wk8"], wk8d, d0)
            if hg == 0:
                # priority-ordered issue: first-needed data first.
                for cp in range(NB_CP):
                    nc.sync.dma_start(out=sb_x8[cp], in_=x8d[cp])
                for cp in range(NB_CP):
                    nc.sync.dma_start(out=sb_x8o[cp], in_=x8od[cp])
                dma_wg(st["wv8"], wv8d, d0)
                load_consts()
            else:
                dma_wg(st["wv8"], wv8d, d0)
            st["q_t"] = {
                pc: p_qk.tile([P, T], bf16, tag=f"q{pc - pcs[0]}", bufs=2,
                              name=f"q{pc}")
                for pc in pcs
            }
            st["k_t"] = {
                pc: p_qk.tile([P, OT], bf16, tag=f"k{pc - pcs[0]}", bufs=2,
                              name=f"k{pc}")
                for pc in pcs
            }
            # v for 4 heads per s-PAIR (256 tokens): [128, 4, 2, 128] fp8
            # (col 64 of each head-slot = 1.0: softmax-denominator row;
            # col 65 = 0.0 pad for an even DoubleRow weight plane; the
            # 128 inner extent keeps the j-plane stride 64B-aligned,
            # which the LDWEIGHTS ISA requires)
            st["v8"] = []
            for sp in range(T // (2 * P)):
                vt = p_v.tile([P, HG, 2, 128], fp8, tag=f"v8_{sp}", bufs=2,
                              name=f"v8_{hg}_{sp}")
                nc.vector.memset(vt[:, :, :, 64:65], 1.0)
                nc.vector.memset(vt[:, :, :, 65:66], 0.0)
                st["v8"].append(vt)
            return st

        def proj_gen(st):
            """Projections (fp8 DoubleRow, contraction 256), one PE chain
            per quantum. Also drains pending softmax normalizes."""
            pcs = st["pcs"]
            for pc in pcs:
                pr = pc - pcs[0]          # 0 or 1 within the group
                for t4 in range(T // 512):
                    ps = p_ps.tile([P, 512], f32, tag="ps", bufs=2,
                                   name="ps_prj")
                    for cp in range(NB_CP):
                        nc.tensor.matmul(
                            ps,
                            st["wq8"][:, cp, :, pr * P : (pr + 1) * P],
                            sb_x8[cp][:, :, t4 * 512 : (t4 + 1) * 512],
                            start=(cp == 0),
                            stop=(cp == NB_CP - 1),
                            perf_mode=PM.DoubleRow,
                        )
                    # unscale the x16 fp8 weights on the way out (DVE: ACT
                    # is the attention-phase bottleneck engine)
                    nc.vector.tensor_scalar_mul(
                        st["q_t"][pc][:, t4 * 512 : (t4 + 1) * 512], ps,
                        1.0 / WSCL,
                    )
                    yield
                for t2 in range(OT // 512):
                    ps = p_ps.tile([P, 512], f32, tag="ps", bufs=2,
                                   name="ps_prk")
                    for cp in range(NB_CP):
                        nc.tensor.matmul(
                            ps,
                            st["wk8"][:, cp, :, pr * P : (pr + 1) * P],
                            sb_x8o[cp][:, :, t2 * 512 : (t2 + 1) * 512],
                            start=(cp == 0),
                            stop=(cp == NB_CP - 1),
                            perf_mode=PM.DoubleRow,
                        )
                    nc.vector.tensor_scalar_mul(
                        st["k_t"][pc][:, t2 * 512 : (t2 + 1) * 512], ps,
                        1.0 / WSCL,
                    )
                    yield
            # deferred softmax normalize of the previous group: its
            # reciprocals are long done by now, so the broadcast matmuls
            # slot into the stream without stalling PE
            yield from apply_norm_gen(p_ps, "ps")
            for sc in range(T // P):
                ps = p_ps.tile([P, HG * D], f32, tag="ps", bufs=2,
                               name="ps_v")
                for cp in range(NB_CP):
                    nc.tensor.matmul(
                        ps,
                        sb_x8[cp][:, :, sc * P : (sc + 1) * P],
                        st["wv8"][:, cp, :, :],
                        start=(cp == 0),
                        stop=(cp == NB_CP - 1),
                        perf_mode=PM.DoubleRow,
                    )
                nc.vector.tensor_scalar_mul(
                    st["v8"][sc // 2][:, :, sc % 2, 0:64],
                    ps[:].rearrange("p (h d) -> p h d", h=HG),
                    1.0 / WSCL,
                )
                yield

        def attn_gen(st):
            pcs, heads = st["pcs"], st["heads"]
            q_t, k_t, v8 = st["q_t"], st["k_t"], st["v8"]
            for tb in range(2):
                nsc = 8 * tb + 8          # s-chunks for this own-block
                av = {}
                for h in heads:
                    av[h] = p_ps.tile(
                        [P, 512], f32, tag=f"av{h % HG}", name=f"av{h}"
                    )
                et_cur = {}
                for sc in range(nsc):
                    c0 = max(0, 64 * sc - 512 * tb)   # first live t-col
                    sp, jj = sc // 2, sc % 2
                    c0p = max(0, 128 * sp - 512 * tb)  # pair's live start
                    for pc in pcs:
                        pr = pc - pcs[0]
                        ps = p_ps.tile(
                            [P, 2, 512], f32, tag="ps", bufs=2, name="ps_sc"
                        )
                        for par in range(2):
                            nc.tensor.matmul(
                                ps[:, par, c0:512],
                                q_t[pc][par * 64 : par * 64 + 64,
                                        sc * P : (sc + 1) * P],
                                k_t[pc][par * 64 : par * 64 + 64,
                                        tb * 512 + c0 : (tb + 1) * 512],
                                start=True,
                                stop=True,
                            )
                        # exp -> fp8, into the (par, j) slice of the pair's
                        # e-tile (allocated at the even chunk)
                        if jj == 0:
                            et_cur[pc] = p_e.tile(
                                [P, 2, 2, 512], fp8, tag="exp",
                                bufs=2, name="et",
                            )
                        et = et_cur[pc]
                        nc.scalar.activation(
                            out=et[:, :, jj, c0:512],
                            in_=ps[:, :, c0:512],
                            func=AF.Exp,
                            scale=SCALE,
                        )
                        if sc >= 8 * tb:   # causal boundary stripe
                            nc.vector.tensor_tensor(
                                et[:, :, jj, c0 : c0 + 64],
                                et[:, :, jj, c0 : c0 + 64],
                                msk2[:, :, 0:64],
                                ALU.mult,
                            )
                            if jj == 1:
                                # odd chunk: the pair-level AV also reads
                                # [c0p, c0) of this j-slice, which exp did
                                # not write this round. memset (a
                                # multiplicative mask can't clean NaN
                                # garbage: NaN*0=NaN).
                                nc.vector.memset(
                                    et[:, :, 1, c0p : c0p + 64], 0.0
                                )
                        if jj == 1:
                            # pair complete: one DoubleRow AV matmul
                            # contracts both 128-token chunks
                            for par in range(2):
                                h = 2 * pc + par
                                hj = heads.index(h)
                                nc.tensor.matmul(
                                    av[h][0:66, c0p:512],
                                    v8[sp][:, hj, :, 0:66],
                                    et[:, par, :, c0p:512],
                                    start=(sp == 0),
                                    stop=(sp == nsc // 2 - 1),
                                    perf_mode=PM.DoubleRow,
                                )
                        yield
                # ---- normalize part 1 (extract): free the av banks fast
                # -- bf16 copies of the value rows (ACT) + denominator
                # rows gathered into 32-aligned partitions of one tile
                # (DVE), one batched reciprocal for all 4 heads. Part 2
                # (broadcast+multiply) is deferred into the next group's
                # projection stream via apply_norm_gen.
                den4 = p_dn.tile([P, 512], f32, tag="den4", bufs=2,
                                 name="den4")
                av_sb = {}
                for hj, h in enumerate(heads):
                    av_sb[h] = p_dn.tile(
                        [64, 512], bf16, tag=f"av_sb{hj}", bufs=2,
                        name=f"av_sb{hj}",
                    )
                    nc.scalar.copy(av_sb[h], av[h][0:64, 0:512])
                    nc.vector.tensor_copy(
                        den4[32 * hj : 32 * hj + 1, :],
                        av[h][64:65, 0:512],
                    )
                rec4 = p_dn.tile([P, 512], f32r, tag="rec4", bufs=2,
                                 name="rec4")
                with nc.allow_low_precision(
                    reason="f32r recip for 1cyc/row bcast"
                ):
                    nc.vector.reciprocal(rec4, den4)
                # matmul reads only allow base partitions {0,32,64}:
                # bounce head 3's reciprocal row down to partition 0
                rec_h3 = p_dn.tile([1, 512], f32r, tag="rec_h3", bufs=2,
                                   name="rec_h3")
                with nc.allow_low_precision(reason="f32r row bounce"):
                    nc.vector.tensor_copy(rec_h3, rec4[96:97, :])
                pending_norm.append(
                    dict(heads=list(heads), tb=tb, av_sb=av_sb,
                         rec4=rec4, rec_h3=rec_h3)
                )
                yield

        # ---- drive: group 0 projections standalone, then interleave ----
        states = {0: prep_group(0)}
        _sc = nc.enter_named_scope("proj0", False)
        for _ in proj_gen(states[0]):
            pass
        nc.leave_named_scope("proj0", _sc[0], False)
        nc.sync.dma_start(out=sb_w1, in_=w1_t)
        nc.sync.dma_start(out=sb_w2, in_=w2_t)

        for hg in range(N_HG):
            _sc = nc.enter_named_scope(f"attn{hg}", False)
            ag = attn_gen(states[hg])
            if hg + 1 < N_HG:
                states[hg + 1] = prep_group(hg + 1)
                pg = proj_gen(states[hg + 1])
            else:
                # last group: no next projections -- still flush the
                # pending normalizes inside the stream so their rotation
                # buffers free before this group's own extractions
                pg = apply_norm_gen(p_ps, "ps")
            done_a = done_p = False
            while not (done_a and done_p):
                for _ in range(2):
                    if not done_a:
                        try:
                            next(ag)
                        except StopIteration:
                            done_a = True
                if not done_p:
                    try:
                        next(pg)
                    except StopIteration:
                        done_p = True
            nc.leave_named_scope(f"attn{hg}", _sc[0], False)

        gen_stack.close()
        xt_stack.close()   # free the fp8 x tiles before the MLP pools open

        # ============================================================
        # Phase B: residual + LN1 + MLP + residual + LN2, per tb
        # ============================================================
        with contextlib.ExitStack() as mctx:
            mctx.enter_context(nc.named_scope("mlp"))
            p_r1 = mctx.enter_context(tc.tile_pool(name="r1", bufs=1))
            p_ln = mctx.enter_context(tc.tile_pool(name="ln", bufs=1))
            p_tmp = mctx.enter_context(tc.tile_pool(name="tmp", bufs=1))
            p_st = mctx.enter_context(tc.tile_pool(name="st", bufs=1))
            p_psm = mctx.enter_context(
                tc.tile_pool(name="psm", bufs=1, space="PSUM")
            )
            p_h = mctx.enter_context(tc.tile_pool(name="hsb", bufs=1))
            p_out = mctx.enter_context(tc.tile_pool(name="outp", bufs=1))

            # bf16 own-token x^T for the attention residual: only read
            # here, so it loads at MLP start (hidden behind apply_norm/LN1)
            sb_xo16 = p_res.tile([P, NB_C, OT], bf16, tag="xo16", name="xo16")
            nc.sync.dma_start(out=sb_xo16, in_=xo16d[:])

            # r1 holds: LN1 input (x + a), then is reused for the pre-LN2
            # residual (ln1 + mlp). All bf16: every LN-chain DVE op is then
            # 2-byte + SBUF-only (4x DVE fast path).
            r1 = [
                p_r1.tile([P, OT], bf16, tag=f"r1_{c}", name=f"r1_{c}")
                for c in range(NB_C)
            ]
            ln1 = [
                p_ln.tile([P, OT], bf16, tag=f"ln1_{c}", name=f"ln1_{c}")
                for c in range(NB_C)
            ]

            def layer_norm_T(src_tiles, gt, bt, out_cb):
                """transposed LN over the partition (c) dim: stats via
                bf16 ones-matmuls, broadcast back via rank-1 matmuls.
                src_tiles: 6 x [128, 512] bf16 views."""
                mu_ps = p_psm.tile([1, 512], f32, tag="lnst", bufs=2, name="mu_ps")
                sq_ps = p_psm.tile([1, 512], f32, tag="lnst", bufs=2, name="sq_ps")
                for c in range(NB_C):
                    s = p_tmp.tile([P, 512], bf16, tag="sqt", bufs=2, name="sqt")
                    nc.vector.tensor_tensor(s, src_tiles[c], src_tiles[c], ALU.mult)
                    nc.tensor.matmul(
                        mu_ps, ones_k, src_tiles[c],
                        start=(c == 0), stop=(c == NB_C - 1),
                    )
                    nc.tensor.matmul(
                        sq_ps, ones_k, s,
                        start=(c == 0), stop=(c == NB_C - 1),
                    )
                mu = p_st.tile([1, 512], f32r, tag="mu_s", bufs=2, name="mu")
                with nc.allow_low_precision(reason="f32r stats for 1cyc/row bcast"):
                    nc.vector.tensor_scalar_mul(mu, mu_ps, 1.0 / C)
                sq = p_st.tile([1, 512], f32, tag="sq_s", bufs=2, name="sq")
                nc.vector.tensor_scalar_mul(sq, sq_ps, 1.0 / C)
                var = p_st.tile([1, 512], f32, tag="var", bufs=2, name="var")
                nc.vector.tensor_tensor(var, mu, mu, ALU.mult)
                nc.vector.tensor_tensor(var, sq, var, ALU.subtract)
                sd = p_st.tile([1, 512], f32, tag="sd", bufs=2, name="sd")
                nc.scalar.activation(
                    out=sd, in_=var, func=AF.Sqrt, bias=eps_t, scale=1.0
                )
                rsg_r = p_st.tile([1, 512], f32r, tag="rsg_r", bufs=2, name="rsg_r")
                with nc.allow_low_precision(reason="f32r rsig for 1cyc/row bcast"):
                    nc.vector.reciprocal(rsg_r, sd)
                mu_b = p_psm.tile([P, 512], f32, tag="lnbc", bufs=2, name="mu_b")
                nc.tensor.matmul(mu_b, ones_b, mu, start=True, stop=True)
                rs_b = p_psm.tile([P, 512], f32, tag="lnbc", bufs=2, name="rs_b")
                nc.tensor.matmul(rs_b, ones_b, rsg_r, start=True, stop=True)
                mu_bs = p_tmp.tile([P, 512], bf16, tag="mu_bs", bufs=1, name="mu_bs")
                nc.scalar.copy(mu_bs, mu_b)
                rs_bs = p_tmp.tile([P, 512], bf16, tag="rs_bs", bufs=1, name="rs_bs")
                nc.scalar.copy(rs_bs, rs_b)
                for c in range(NB_C):
                    d1 = p_tmp.tile([P, 512], bf16, tag="d1", bufs=2, name="d1")
                    nc.vector.tensor_tensor(d1, src_tiles[c], mu_bs, ALU.subtract)
                    nc.vector.tensor_tensor(d1, d1, rs_bs, ALU.mult)
                    out_cb(c, d1, gt, bt)

            # flush the last attention group's deferred normalize (psum via
            # the h_ps rotation slots, which mlp_pass1 then takes over)
            for _ in apply_norm_gen(p_psm, "h_ps"):
                pass

            # residual + LN1 for BOTH halves first: LN1(tb=1)'s DVE work
            # then overlaps MLP(tb=0)'s matmuls.
            for tb in range(2):
                sl = slice(tb * 512, (tb + 1) * 512)
                r1v = []
                for c in range(NB_C):
                    nc.vector.tensor_tensor(
                        r1[c][:, sl], sb_xo16[:, c, sl], sb_a[c][:, sl], ALU.add
                    )
                    r1v.append(r1[c][:, sl])

                def ln1_out(c, d2, gt, bt, _sl=sl):
                    nc.vector.tensor_scalar(
                        out=ln1[c][:, _sl], in0=d2,
                        scalar1=gt[:, c : c + 1], scalar2=bt[:, c : c + 1],
                        op0=ALU.mult, op1=ALU.add,
                    )

                layer_norm_T(r1v, sb_g1, sb_be1, ln1_out)

            def mlp_pass1(tb):
                sl = slice(tb * 512, (tb + 1) * 512)
                h_sb = []
                for m in range(NB_F):
                    h_ps = p_psm.tile([P, 512], f32, tag="h_ps", bufs=2, name="h_ps")
                    for k in range(NB_C):
                        nc.tensor.matmul(
                            h_ps,
                            sb_w1[:, k, m * P : (m + 1) * P],
                            ln1[k][:, sl],
                            start=(k == 0),
                            stop=(k == NB_C - 1),
                        )
                    hs = p_h.tile([P, 512], bf16, tag=f"h{m}", name=f"h{m}")
                    nc.scalar.activation(
                        out=hs, in_=h_ps, func=AF.Gelu,
                        bias=sb_b1[:, m : m + 1], scale=1.0,
                    )
                    h_sb.append(hs)
                return h_sb

            def mlp_pass2(tb, h_sb):
                sl = slice(tb * 512, (tb + 1) * 512)
                r2v = []
                for c in range(NB_C):
                    y_ps = p_psm.tile([P, 512], f32, tag="y_ps", bufs=2, name="y_ps")
                    for m in range(NB_F):
                        nc.tensor.matmul(
                            y_ps,
                            sb_w2[:, m, c * P : (c + 1) * P],
                            h_sb[m],
                            start=(m == 0),
                            stop=(m == NB_F - 1),
                        )
                    y_sb = p_h.tile([P, 512], bf16, tag="y_sb", bufs=2, name="y_sb")
                    nc.scalar.activation(
                        out=y_sb, in_=y_ps, func=AF.Identity,
                        bias=sb_b2[:, c : c + 1], scale=1.0,
                    )
                    # pre-LN2 residual, reusing r1 (its LN1-input role is done)
                    nc.vector.tensor_tensor(
                        r1[c][:, sl], y_sb, ln1[c][:, sl], ALU.add
                    )
                    r2v.append(r1[c][:, sl])
                return r2v

            def ln2(tb, r2v):
                sl = slice(tb * 512, (tb + 1) * 512)

                def ln2_out(c, d2, gt, bt, _sl=sl):
                    o = p_out.tile([P, 512], f32, tag="o", bufs=2, name="o")
                    nc.vector.tensor_scalar(
                        out=o, in0=d2,
                        scalar1=gt[:, c : c + 1], scalar2=bt[:, c : c + 1],
                        op0=ALU.mult, op1=ALU.add,
                    )
                    nc.sync.dma_start(out=outT_t[c][:, _sl], in_=o)

                layer_norm_T(r2v, sb_g2, sb_be2, ln2_out)

            # software-pipelined: PE charges into pass1(1) while the
            # DVE/ACT tail of ln2(0) drains.
            h0 = mlp_pass1(0)
            r2_0 = mlp_pass2(0, h0)
            h1 = mlp_pass1(1)
            ln2(0, r2_0)
            r2_1 = mlp_pass2(1, h1)
            ln2(1, r2_1)

    return nc


def _spill_excess_waits(nc, maxw=2):
    """walrus (this build) caps sync-wait commands per instruction. Move
    excess waits onto freshly inserted same-engine nops placed immediately
    before the over-limit instruction (same engine stream => the waits
    still complete before it executes)."""
    import copy

    import concourse.bass as bass
    import concourse.mybir as mybir

    scratch = bass.Bass()
    tpl = scratch.sync.nop(nofuse=True).ins
    ctr = [0]

    def mknop(engine, waits):
        n = copy.deepcopy(tpl)
        ctr[0] += 1
        n.name = f"I-spill{ctr[0]}"
        n.engine = engine
        n.sync_info = mybir.SyncInfo(on_wait=list(waits), on_update=[])
        return n

    fn = nc.m.functions[0]
    for bb in fn.blocks:
        changed = False
        out = []
        for inst in bb.instructions:
            si = inst.sync_info
            waits = list(si.on_wait) if si and si.on_wait else []
            nupd = len(si.on_update) if si and si.on_update else 0
            lim = max(0, maxw - nupd)   # waits + updates <= maxw total
            if len(waits) > lim:
                keep = waits[-lim:] if lim else []
                rest = waits[: len(waits) - lim]
                while rest:
                    chunk, rest = rest[:1], rest[1:]
                    out.append(mknop(inst.engine, chunk))
                si.on_wait = keep
                changed = True
            out.append(inst)
        if changed:
            bb.instructions = out


def _get_nc():
    if "nc" not in _compiled:
        _patch_tile_drain()
        _patch_profile_hook()
        nc = _build_nc()
        _spill_excess_waits(nc, maxw=2)
        _compiled["nc"] = nc
    return _compiled["nc"]


# --------------------------------------------------------------------------
# host-side sharding
# --------------------------------------------------------------------------

def _cpair8(a):
    """[C, N] f32 -> [NB_CP, 128, 2, N] fp8 (c-pair DoubleRow layout)."""
    n = a.shape[1]
    return np.ascontiguousarray(
        a.reshape(NB_CP, 2, P, n).transpose(0, 2, 1, 3)
    ).astype(E4M3)


def _w8(a):
    """[C, N] f32 -> [128, NB_CP, 2, N] fp8 (weight DoubleRow layout)."""
    n = a.shape[1]
    return np.ascontiguousarray(
        a.reshape(NB_CP, 2, P, n).transpose(2, 0, 1, 3)
    ).astype(E4M3)


def _make_in_maps(x, Wq, Wk, Wv, ln1_g, ln1_b, W1, b1, W2, b2, ln2_g, ln2_b):
    x = np.asarray(x, np.float32)
    wq_s = np.ascontiguousarray(
        np.asarray(Wq, np.float32).transpose(1, 0, 2).reshape(C, C)
    )
    wk_s = np.ascontiguousarray(
        np.asarray(Wk, np.float32).transpose(1, 0, 2).reshape(C, C)
    )
    wv_s = np.ascontiguousarray(
        np.asarray(Wv, np.float32).transpose(1, 0, 2).reshape(C, C)
    )
    wq8 = _w8(wq_s * np.float32(WSCL))
    wk8 = _w8(wk_s * np.float32(WSCL))
    wv8 = _w8(wv_s * np.float32(WSCL))
    w1b = np.asarray(W1, np.float32).astype(BF16)
    w2b = np.asarray(W2, np.float32).astype(BF16)
    b1r = np.ascontiguousarray(np.asarray(b1, np.float32).reshape(NB_F, P).T)
    b2r = np.ascontiguousarray(np.asarray(b2, np.float32).reshape(NB_C, P).T)
    g1r = np.ascontiguousarray(np.asarray(ln1_g, np.float32).reshape(NB_C, P).T)
    be1r = np.ascontiguousarray(np.asarray(ln1_b, np.float32).reshape(NB_C, P).T)
    g2r = np.ascontiguousarray(np.asarray(ln2_g, np.float32).reshape(NB_C, P).T)
    be2r = np.ascontiguousarray(np.asarray(ln2_b, np.float32).reshape(NB_C, P).T)

    in_maps = []
    for core in range(N_CORES):
        b, g = core // 2, core % 2
        xb = x[b]                                # [T, C]
        xTa = np.ascontiguousarray(xb.T)         # [C, T]
        own = np.arange(g, T, 2)
        xo = np.ascontiguousarray(xb[own].T)     # [C, OT]
        ii = np.arange(P)[:, None]
        mm = np.arange(64)[None, :]
        cm = np.where(ii <= 2 * mm + g, 1.0, 0.0).astype(BF16)
        cm2 = np.concatenate(
            [np.zeros((P, 64), np.float32), cm.astype(np.float32)], axis=1
        ).astype(BF16)
        xo16 = np.ascontiguousarray(
            xo.reshape(NB_C, P, OT).transpose(1, 0, 2)
        ).astype(BF16)
        in_maps.append(
            {
                "x8d": _cpair8(xTa),
                "x8od": _cpair8(xo),
                "xo16d": xo16,
                "wq8d": wq8,
                "wk8d": wk8,
                "wv8d": wv8,
                "w1d": w1b,
                "w2d": w2b,
                "b1r": b1r,
                "b2r": b2r,
                "g1r": g1r,
                "be1r": be1r,
                "g2r": g2r,
                "be2r": be2r,
                "cmask": cm,
                "cmask2": cm2,
            }
        )
    return in_maps


def _assemble(results):
    out = np.empty((B, T, C), np.float32)
    for core in range(N_CORES):
        b, g = core // 2, core % 2
        own = np.arange(g, T, 2)
        out[b, own, :] = results[core]["outT"].T
    return out


def kernel(_trace=False, **inputs):
    from concourse.bass_utils import run_bass_kernel_spmd

    nc = _get_nc()
    in_maps = _make_in_maps(**inputs)
    res = run_bass_kernel_spmd(nc, in_maps, list(range(N_CORES)), trace=_trace)
    out = _assemble(res.results)
    if _trace:
        return out, res
    return out
